# revision 30
# baseline (speedup 1.0000x reference)
"""Trainium2 Bass kernel for nn_Attention_3728031613575.

Multi-head attention, B=4 L=2048 D=1024 H=16 (head dim 64), fp32 reference:
    q/k/v = split_heads(x @ W{q,k,v} + b)        [b,h,l,64]
    scores = q k^T + mask * (-1e5)
    out    = softmax(scores) @ v                 -> [b,l,1024]

Sharding (8 cores): core c handles batch b = c//2 and heads (c%2)*8..+8
(batch x head-group data parallel; QKV weights column-sharded by head).
Attention is fully local per core; no collectives.

Per-core algorithm (layouts chosen so softmax lives on the PSUM partition
dim and no probability transposes are ever needed):
  - X^T built once via PE transposes (fp32r), evacuated on the Pool engine.
  - Q^T/K^T [head-dims, l] and V [l, head-dims] projections in fp32r;
    Q/K biases folded into the Pool-engine PSUM evacuation as a
    per-partition tensor_scalar_add; V bias as a rank-1 matmul term.
  - mask preprocessed once per core:  M_e = exp(-1e5*(m - rowmin(m)))
    (the rowmin bias provides exact max-subtraction for the mask-dominated
    term; the remaining q.k part is range-safe in fp32), stored bf16,
    reloaded transposed through the DMA xbar on the SP queue.
  - per (head-pair, q-block 512, k-block 128):
      S^T[k,q] = K^T.T @ Q^T        (two K=64 matmuls row-tiled on the PE)
      E = exp(S^T)                  (ACT, bf16, from PSUM)
      P~ = E * M_e^T                (DVE, bf16, 2-kb-wide ops)
      O'^T[d,q] += V^T P~ with a ones-column in V producing the softmax
      denominators as row 64 of O'.
  - postproc: PE-transpose O'^T -> [q, 65] (f32r datapath), reciprocal of
    col 64, tensor_scalar normalize, DMA out.

Scheduling: the QKV projection is split into fine-grained work items
(one transpose pair / one 8-matmul projection chain each) drained from a
pump queue between attention k-blocks, so the PE never executes a long
projection blob while the ACT exp stream starves.  The mask pipeline
(DMA in -> DVE rowmin -> ACT exp -> DMA out -> transposed met loads) is
pumped the same way; met tiles for q-block qb+1 prefetch during qb.
Evacuation copies run on the otherwise-idle Pool engine.
"""

import os
import sys

sys.path.insert(0, "/opt/trn_rl_repo")

import numpy as np

B, L, D, H, DH = 4, 2048, 1024, 16, 64
NCORES = 8
HPC = 8            # heads per core
NPAIR = HPC // 2   # head pairs per core
QBW = 512          # q block width
NQB = L // QBW     # 4 q blocks
NKB = L // 128     # 16 k blocks
NDB = D // 128     # 8 contraction chunks
MASK_C = -100000.0

_CACHE = {}


def _build():
    import concourse.bass as bass
    from concourse import bacc, mybir
    import concourse.tile as tile
    from concourse.masks import make_identity

    F32 = mybir.dt.float32
    F32R = mybir.dt.float32r
    BF16 = mybir.dt.bfloat16
    AF = mybir.ActivationFunctionType
    ALU = mybir.AluOpType
    AX = mybir.AxisListType

    nc = bacc.Bacc(None, target_bir_lowering=False)

    x_d = nc.dram_tensor("x", [L, D], F32R, kind="ExternalInput")
    mask_d = nc.dram_tensor("mask", [L, L], F32, kind="ExternalInput")
    wq_d = nc.dram_tensor("wq", [D, 512], F32R, kind="ExternalInput")
    wk_d = nc.dram_tensor("wk", [D, 512], F32R, kind="ExternalInput")
    wv_d = nc.dram_tensor("wv", [D, 512], F32R, kind="ExternalInput")
    bq_d = nc.dram_tensor("bq", [1, 512], F32, kind="ExternalInput")
    bk_d = nc.dram_tensor("bk", [1, 512], F32, kind="ExternalInput")
    bv_d = nc.dram_tensor("bv", [1, 512], F32R, kind="ExternalInput")
    out_d = nc.dram_tensor("out", [L, 512], F32, kind="ExternalOutput")

    with tile.TileContext(nc) as tc:
        with tc.tile_pool(name="const", bufs=1) as constp, \
             tc.tile_pool(name="persist", bufs=1) as pers, \
             tc.tile_pool(name="dram", bufs=1, space="DRAM") as dramp, \
             tc.tile_pool(name="met", bufs=4) as metp, \
             tc.tile_pool(name="stage", bufs=1) as stagep, \
             tc.tile_pool(name="epool", bufs=3) as epool, \
             tc.tile_pool(name="oevac", bufs=2) as oevacp, \
             tc.tile_pool(name="rpool", bufs=4) as rpool, \
             tc.tile_pool(name="wpool", bufs=1) as wpool, \
             tc.tile_pool(name="xload", bufs=3) as xload, \
             tc.tile_pool(name="xtpool", bufs=1) as xtpool, \
             tc.tile_pool(name="mload", bufs=2) as mload, \
             tc.tile_pool(name="mtmp", bufs=2) as mtmp, \
             tc.tile_pool(name="spsum", bufs=2, space="PSUM") as spsum, \
             tc.tile_pool(name="opsum", bufs=2, space="PSUM") as opsum, \
             tc.tile_pool(name="scratch", bufs=2, space="PSUM") as scratch:

            # ---- constants
            idf32 = constp.tile([128, 128], F32, name="idf32", tag="idf32")
            make_identity(nc, idf32)
            idf32r = constp.tile([128, 128], F32R, name="idf32r", tag="idf32r")
            nc.vector.tensor_copy(idf32r, idf32)
            ones_f = constp.tile([1, 128], F32, name="ones_f", tag="ones_f")
            nc.vector.memset(ones_f, 1.0)
            ones_r = constp.tile([1, 128], F32R, name="ones_r", tag="ones_r")
            nc.vector.tensor_copy(ones_r, ones_f)

            # ---- persistent activations
            QT = pers.tile([128, NPAIR, L], F32R, name="QT", tag="QT")
            KT = pers.tile([128, NPAIR, L], F32R, name="KT", tag="KT")
            V = pers.tile([128, NKB, HPC, DH + 1], BF16, name="V", tag="V")
            nc.vector.memset(V[:, :, :, DH], 1.0)

            me_dram = dramp.tile([L, L], BF16, name="me_dram", tag="me_dram")

            # ---- weight / bias loads (DMA queue order matters: wk+wv feed
            # the pre-attention projections; masks for qb0 go next; wq is
            # only needed from qb1 on)

            xts = {}

            def xt_tile(lb):
                if lb not in xts:
                    xts[lb] = xtpool.tile([128, NDB, QBW], F32R,
                                          name="xt%d" % lb, tag="xt")
                return xts[lb]

            # ---------------- mask pipeline (half-width tiles) -----------
            def emit_mask_load(qb):
                mls = []
                for h in range(2):
                    ml = mload.tile([128, L // 2], F32,
                                    name=f"ml{qb}_{h}", tag="ml")
                    nc.sync.dma_start(
                        out=ml,
                        in_=mask_d[qb * 128:(qb + 1) * 128,
                                   h * (L // 2):(h + 1) * (L // 2)])
                    mls.append(ml)
                return mls

            def emit_mask_exp(qb, mls):
                mm = rpool.tile([128, 2], F32, name=f"mm{qb}", tag="rc")
                for h in range(2):
                    nc.vector.tensor_reduce(mm[:, h:h + 1], mls[h],
                                            axis=AX.X, op=ALU.min)
                mbias = rpool.tile([128, 1], F32, name=f"mb{qb}", tag="rc")
                nc.vector.tensor_reduce(mbias, mm, axis=AX.X, op=ALU.min)
                nc.vector.tensor_scalar_mul(mbias, mbias, -MASK_C)
                for h in range(2):
                    me = mtmp.tile([128, L // 2], BF16,
                                   name=f"me{qb}_{h}", tag="me")
                    nc.scalar.activation(me, mls[h], AF.Exp,
                                         bias=mbias, scale=MASK_C)
                    nc.sync.dma_start(
                        out=me_dram[qb * 128:(qb + 1) * 128,
                                    h * (L // 2):(h + 1) * (L // 2)],
                        in_=me)

            met_tiles = {}

            def emit_met(qb_, c):
                """Transposed M_e tiles for (q block, k-block chunk c)."""
                q0 = qb_ * QBW
                mh = metp.tile([128, 4, QBW], BF16,
                               name=f"met{qb_}_{c}", tag="met")
                met_tiles[(qb_, c)] = mh
                for i in range(4):
                    nc.sync.dma_start_transpose(
                        mh[:, i, :],
                        me_dram[q0:q0 + QBW,
                                (4 * c + i) * 128:(4 * c + i + 1) * 128])

            # ---------------- projection work items ---------------------
            xload_tiles = {}

            def emit_xt_load(lb, sh):
                for s in range(2):
                    xl = xload.tile([128, D], F32R,
                                    name=f"xl{lb}_{sh}_{s}", tag="xl")
                    nc.sync.dma_start(
                        out=xl,
                        in_=x_d[lb * 512 + (sh * 2 + s) * 128:
                                lb * 512 + (sh * 2 + s + 1) * 128, :])
                    xload_tiles[(lb, sh, s)] = xl

            def emit_xt(lb, sh):
                """Transpose X rows [lb*512+sh*256, +256) into xt."""
                xt = xt_tile(lb)
                if (lb, sh, 0) not in xload_tiles:
                    emit_xt_load(lb, sh)
                xls = [xload_tiles.pop((lb, sh, s)) for s in range(2)]
                for db in range(NDB):
                    tpt = scratch.tile([128, 256], F32R,
                                       name=f"tpd{lb}_{sh}_{db}", tag="sc")
                    for s in range(2):
                        nc.tensor.transpose(
                            tpt[:, s * 128:(s + 1) * 128],
                            xls[s][:, db * 128:(db + 1) * 128],
                            idf32r)
                    nc.vector.tensor_copy(
                        xt[:, db, sh * 256:(sh + 1) * 256], tpt)

            def emit_qk_chain(w_sb, bias_t, dst, np_, lb):
                """One projection chain into dst (KT slice or QT chunk)."""
                qp = scratch.tile([128, 512], F32,
                                  name=f"qp{np_}_{lb}_{id(w_sb) % 97}",
                                  tag="sc")
                for db in range(NDB):
                    nc.tensor.matmul(
                        qp,
                        w_sb[:, db, np_ * 128:(np_ + 1) * 128],
                        xt_tile(lb)[:, db, :],
                        start=(db == 0), stop=(db == NDB - 1))
                nc.vector.tensor_scalar_add(
                    dst[:, np_, lb * 512:(lb + 1) * 512], qp,
                    bias_t[:, np_:np_ + 1])

            def emit_v_chain(kb):
                vp = scratch.tile([128, 512], F32, name=f"vp{kb}", tag="sc")
                for db in range(NDB):
                    nc.tensor.matmul(
                        vp,
                        xt_tile(kb // 4)[:, db, (kb % 4) * 128:
                                         (kb % 4 + 1) * 128],
                        wv[:, db, :],
                        start=(db == 0), stop=False)
                nc.tensor.matmul(vp, ones_r[0:1, 0:128], bv,
                                 start=False, stop=True)
                nc.vector.tensor_copy(
                    V[:, kb, :, 0:DH],
                    vp.rearrange("p (h d) -> p h d", h=HPC))

            # ---------------- pump ---------------------------------------
            work = []

            def pump(n):
                for _ in range(n):
                    if work:
                        work.pop(0)()

            # ---------------- attention unit ------------------------------
            def emit_attn_pair(qb_, pr, rate=2):
                hA, hB = 2 * pr, 2 * pr + 1
                q0 = qb_ * QBW
                oa = opsum.tile([DH + 1, QBW], F32,
                                name=f"oa{qb_}_{pr}", tag="o")
                ob = opsum.tile([DH + 1, QBW], F32,
                                name=f"ob{qb_}_{pr}", tag="o")
                eps = {}

                def emit_ttpv(g):
                    """mask-multiply + PV for kb group (2g, 2g+1) — emitted
                    ~2 k-blocks late so the PE's in-order queue never parks
                    a PV (waiting on exp) in front of a ready QK."""
                    ppg = eps[g].rearrange("p a (b f) -> p a b f", b=2)
                    mh = met_tiles[(qb_, g // 2)]
                    base = mh[:, (2 * g) % 4, :]
                    mdup = bass.AP(
                        tensor=mh.tensor,
                        offset=base.offset,
                        ap=[mh.ap[0], [QBW, 2], [0, 2], [1, QBW]])
                    nc.vector.tensor_tensor(
                        out=ppg, in0=ppg, in1=mdup, op=ALU.mult)
                    for dkb in (2 * g, 2 * g + 1):
                        for o_ps, h, half in ((oa, hA, 0), (ob, hB, 1)):
                            nc.tensor.matmul(
                                o_ps,
                                V[:, dkb, h, :],
                                ppg[:, dkb % 2, half, :],
                                start=(dkb == 0),
                                stop=(dkb == NKB - 1))

                for kb in range(NKB):
                    if pr == 0 and kb == 0:
                        for c in range(4):
                            emit_met(qb_, c)
                    sp = spsum.tile([128, 1024], F32,
                                    name=f"sp{qb_}_{pr}_{kb}", tag="s")
                    nc.tensor.matmul(
                        sp[:, 0:512],
                        KT[0:64, pr, kb * 128:(kb + 1) * 128],
                        QT[0:64, pr, q0:q0 + QBW],
                        start=True, stop=True, tile_position=(0, 0))
                    nc.tensor.matmul(
                        sp[:, 512:1024],
                        KT[64:128, pr, kb * 128:(kb + 1) * 128],
                        QT[64:128, pr, q0:q0 + QBW],
                        start=True, stop=True, tile_position=(64, 0))
                    if kb % 2 == 0:
                        eps[kb // 2] = epool.tile(
                            [128, 2, 1024], BF16,
                            name=f"e{qb_}_{pr}_{kb}", tag="e")
                    nc.scalar.activation(eps[kb // 2][:, kb % 2, :],
                                         sp, AF.Exp)
                    if kb % 2 == 1:
                        pump(rate)
                        if kb >= 3:
                            emit_ttpv((kb - 3) // 2)
                def postproc():
                    stage = stagep.tile([128, 4, 128], F32,
                                        name=f"st{qb_}_{pr}", tag="st")
                    osbA = oevacp.tile([DH + 1, QBW], F32,
                                       name=f"oeA{qb_}_{pr}", tag="oe")
                    osbB = oevacp.tile([DH + 1, QBW], F32,
                                       name=f"oeB{qb_}_{pr}", tag="oe")
                    nc.vector.tensor_copy(osbA, oa)
                    nc.vector.tensor_copy(osbB, ob)
                    for osb, h in ((osbA, hA), (osbB, hB)):
                        hcol = (h % 2) * DH
                        tp = scratch.tile([128, 4 * 65], F32,
                                          name=f"tq{qb_}_{pr}_{h}", tag="sc")
                        for j in range(4):
                            nc.tensor.transpose(
                                tp[:, j * 65:(j + 1) * 65],
                                osb[:, j * 128:(j + 1) * 128],
                                idf32[0:65, 0:65])
                        tpv = tp.rearrange("p (j c) -> p j c", j=4)
                        rec = rpool.tile([128, 4], F32,
                                         name=f"rc{qb_}_{pr}_{h}", tag="rc")
                        nc.vector.reciprocal(rec, tpv[:, :, 64:65])
                        for j in range(4):
                            nc.vector.tensor_scalar_mul(
                                stage[:, j, hcol:hcol + DH],
                                tpv[:, j, 0:DH],
                                rec[:, j:j + 1])
                    nc.sync.dma_start(
                        out=out_d[q0:q0 + QBW, pr * 128:(pr + 1) * 128]
                        .rearrange("(j p) c -> p j c", p=128),
                        in_=stage)

                # defer the final PV group and the postproc to the next
                # unit's first pump call: its early QKs slot in ahead so
                # neither the PV tail nor the postproc transposes ever make
                # the ACT exp stream wait at a unit boundary
                work.insert(0, postproc)
                work.insert(0, lambda: emit_ttpv(NKB // 2 - 1))

            # ---------------- emission schedule ---------------------------
            # DMA queue order: X lb0 (gates everything), wk + bias tiles,
            # qb0's mask blocks (gate the met transposes), wq, wv.  The met
            # transposes for qb0 queue right behind the mask stores, ahead
            # of the pumped mask 4-15 loads.
            emit_xt_load(0, 0)
            emit_xt_load(0, 1)
            wk = wpool.tile([128, NDB, 512], F32R, name="wk", tag="wk")
            nc.sync.dma_start(out=wk,
                              in_=wk_d.rearrange("(c p) n -> p c n", p=128))
            bqt = wpool.tile([128, NPAIR], F32, name="bqt", tag="bqt")
            nc.sync.dma_start(out=bqt,
                              in_=bq_d.rearrange("o (c p) -> (o p) c", p=128))
            bkt = wpool.tile([128, NPAIR], F32, name="bkt", tag="bkt")
            nc.sync.dma_start(out=bkt,
                              in_=bk_d.rearrange("o (c p) -> (o p) c", p=128))
            emit_xt_load(1, 0)
            emit_xt_load(1, 1)
            wq = wpool.tile([128, NDB, 512], F32R, name="wq", tag="wq")
            nc.sync.dma_start(out=wq,
                              in_=wq_d.rearrange("(c p) n -> p c n", p=128))
            wv = wpool.tile([128, NDB, 512], F32R, name="wv", tag="wv")
            nc.sync.dma_start(out=wv,
                              in_=wv_d.rearrange("(c p) n -> p c n", p=128))
            bv = wpool.tile([1, 512], F32R, name="bv", tag="bv")
            nc.sync.dma_start(out=bv, in_=bv_d[:, :])
            mls0 = {qb: emit_mask_load(qb) for qb in range(4)}
            emit_xt(0, 0)
            emit_xt(0, 1)
            for qb in range(4):
                emit_mask_exp(qb, mls0.pop(qb))
            emit_qk_chain(wk, bkt, KT, 0, 0)
            emit_qk_chain(wq, bqt, QT, 0, 0)
            for kb in range(4):
                emit_v_chain(kb)

            def mask_item(qb):
                # exp scheduled a few pump slots after its load so the ACT
                # queue never head-blocks on an in-flight mask DMA
                mlq = emit_mask_load(qb)
                work.insert(min(8, len(work)),
                            lambda: emit_mask_exp(qb, mlq))

            def W(fn, *a):
                work.append(lambda: fn(*a))

            # Backlog drained in unit (0,0) (rate 9): the full projection in
            # l-chunk-major order (the rotating xt tile requires each chunk's
            # 12 consumers emitted before the chunk two slots later rebuilds)
            # plus the remaining mask pipelines, which feed the ACT engine
            # while the PE grinds through projection chains.
            proj = []
            proj += [(emit_qk_chain, wk, bkt, KT, pr, 0) for pr in (1, 2, 3)]
            proj += [(emit_qk_chain, wq, bqt, QT, pr, 0) for pr in (1, 2, 3)]
            late = []
            for lb in range(1, 4):
                if lb < 3:
                    proj.append((emit_xt_load, lb + 1, 0))
                    proj.append((emit_xt_load, lb + 1, 1))
                proj.append((emit_xt, lb, 0))
                proj.append((emit_xt, lb, 1))
                proj += [(emit_v_chain, kb)
                         for kb in range(4 * lb, 4 * lb + 4)]
                if lb < 3:
                    proj += [(emit_qk_chain, wk, bkt, KT, pr, lb)
                             for pr in range(4)]
                    proj += [(emit_qk_chain, wq, bqt, QT, pr, lb)
                             for pr in range(4)]
                else:
                    # the last l-chunk's xt is never overwritten, so its
                    # late-deadline chains can drain in later units' PE
                    # slack instead of stretching the PE-bound window:
                    # K p2/p3 l3 before units (0,2)/(0,3); Q l3 before qb3
                    proj += [(emit_qk_chain, wk, bkt, KT, pr, lb)
                             for pr in (0, 1)]
                    proj.append((emit_qk_chain, wq, bqt, QT, 0, lb))
                    late += [(emit_qk_chain, wk, bkt, KT, 2, lb),
                             (emit_qk_chain, wq, bqt, QT, 1, lb),
                             (emit_qk_chain, wk, bkt, KT, 3, lb),
                             (emit_qk_chain, wq, bqt, QT, 2, lb),
                             (emit_qk_chain, wq, bqt, QT, 3, lb)]
            mqueue = list(range(4, 16))
            for item in proj:
                W(*item)
            for _ in range(6):
                W(mask_item, mqueue.pop(0))
            for item in late:
                W(*item)

            emit_attn_pair(0, 0, rate=8)
            for pr in range(1, NPAIR):
                if mqueue:
                    W(mask_item, mqueue.pop(0))
                    W(mask_item, mqueue.pop(0))
                emit_attn_pair(0, pr, rate=2)
            for qb_ in range(1, NQB):
                for pr in range(NPAIR):
                    if mqueue:
                        W(mask_item, mqueue.pop(0))
                        W(mask_item, mqueue.pop(0))
                    emit_attn_pair(qb_, pr, rate=2)
            pump(len(work))

    nc.finalize()
    return nc


def _get_nc():
    if "nc" not in _CACHE:
        _CACHE["nc"] = _build()
    return _CACHE["nc"]


def kernel(embedding, mask, Wq, bq, Wk, bk, Wv, bv):
    from concourse.bass_utils import run_bass_kernel_spmd

    nc = _get_nc()

    embedding = np.asarray(embedding, dtype=np.float32)
    mask = np.asarray(mask, dtype=np.float32)
    in_maps = []
    for c in range(NCORES):
        b = c // 2
        h0 = (c % 2) * HPC
        cs = slice(h0 * DH, (h0 + HPC) * DH)
        in_maps.append({
            "x": np.ascontiguousarray(embedding[b]),
            "mask": np.ascontiguousarray(mask[b, 0]),
            "wq": np.ascontiguousarray(np.asarray(Wq, np.float32)[:, cs]),
            "wk": np.ascontiguousarray(np.asarray(Wk, np.float32)[:, cs]),
            "wv": np.ascontiguousarray(np.asarray(Wv, np.float32)[:, cs]),
            "bq": np.ascontiguousarray(np.asarray(bq, np.float32)[cs]).reshape(1, 512),
            "bk": np.ascontiguousarray(np.asarray(bk, np.float32)[cs]).reshape(1, 512),
            "bv": np.ascontiguousarray(np.asarray(bv, np.float32)[cs]).reshape(1, 512),
        })

    res = run_bass_kernel_spmd(nc, in_maps, core_ids=list(range(NCORES)))

    out = np.empty((B, L, D), dtype=np.float32)
    for c in range(NCORES):
        b = c // 2
        h0 = (c % 2) * HPC
        out[b][:, h0 * DH:(h0 + HPC) * DH] = res.results[c]["out"]
    return out


# revision 33
# speedup vs baseline: 1.0188x; 1.0188x over previous
"""Trainium2 Bass kernel for nn_Attention_3728031613575.

Multi-head attention, B=4 L=2048 D=1024 H=16 (head dim 64), fp32 reference:
    q/k/v = split_heads(x @ W{q,k,v} + b)        [b,h,l,64]
    scores = q k^T + mask * (-1e5)
    out    = softmax(scores) @ v                 -> [b,l,1024]

Sharding (8 cores): core c handles batch b = c//2 and heads (c%2)*8..+8
(batch x head-group data parallel; QKV weights column-sharded by head).
Attention is fully local per core; no collectives.

Per-core algorithm (layouts chosen so softmax lives on the PSUM partition
dim and no probability transposes are ever needed):
  - X^T built once via PE transposes (fp32r), evacuated on the Pool engine.
  - Q^T/K^T [head-dims, l] and V [l, head-dims] projections in fp32r;
    Q/K biases folded into the Pool-engine PSUM evacuation as a
    per-partition tensor_scalar_add; V bias as a rank-1 matmul term.
  - mask preprocessed once per core:  M_e = exp(-1e5*(m - rowmin(m)))
    (the rowmin bias provides exact max-subtraction for the mask-dominated
    term; the remaining q.k part is range-safe in fp32), stored bf16,
    reloaded transposed through the DMA xbar on the SP queue.
  - per (head-pair, q-block 512, k-block 128):
      S^T[k,q] = K^T.T @ Q^T        (two K=64 matmuls row-tiled on the PE)
      E = exp(S^T)                  (ACT, bf16, from PSUM)
      P~ = E * M_e^T                (DVE, bf16, 2-kb-wide ops)
      O'^T[d,q] += V^T P~ with a ones-column in V producing the softmax
      denominators as row 64 of O'.
  - postproc: PE-transpose O'^T -> [q, 65] (f32r datapath), reciprocal of
    col 64, tensor_scalar normalize, DMA out.

Scheduling: the QKV projection is split into fine-grained work items
(one transpose pair / one 8-matmul projection chain each) drained from a
pump queue between attention k-blocks, so the PE never executes a long
projection blob while the ACT exp stream starves.  The mask pipeline
(DMA in -> DVE rowmin -> ACT exp -> DMA out -> transposed met loads) is
pumped the same way; met tiles for q-block qb+1 prefetch during qb.
Evacuation copies run on the otherwise-idle Pool engine.
"""

import os
import sys

sys.path.insert(0, "/opt/trn_rl_repo")

import numpy as np

B, L, D, H, DH = 4, 2048, 1024, 16, 64
NCORES = 8
HPC = 8            # heads per core
NPAIR = HPC // 2   # head pairs per core
QBW = 512          # q block width
NQB = L // QBW     # 4 q blocks
NKB = L // 128     # 16 k blocks
NDB = D // 128     # 8 contraction chunks
MASK_C = -100000.0

_CACHE = {}


def _build():
    import concourse.bass as bass
    from concourse import bacc, mybir
    import concourse.tile as tile
    from concourse.masks import make_identity

    F32 = mybir.dt.float32
    F32R = mybir.dt.float32r
    BF16 = mybir.dt.bfloat16
    AF = mybir.ActivationFunctionType
    ALU = mybir.AluOpType
    AX = mybir.AxisListType

    nc = bacc.Bacc(None, target_bir_lowering=False)

    x_d = nc.dram_tensor("x", [L, D], F32R, kind="ExternalInput")
    mask_d = nc.dram_tensor("mask", [L, L], F32, kind="ExternalInput")
    wq_d = nc.dram_tensor("wq", [D, 512], F32R, kind="ExternalInput")
    wk_d = nc.dram_tensor("wk", [D, 512], F32R, kind="ExternalInput")
    wv_d = nc.dram_tensor("wv", [D, 512], F32R, kind="ExternalInput")
    bq_d = nc.dram_tensor("bq", [1, 512], F32, kind="ExternalInput")
    bk_d = nc.dram_tensor("bk", [1, 512], F32, kind="ExternalInput")
    bv_d = nc.dram_tensor("bv", [1, 512], F32R, kind="ExternalInput")
    out_d = nc.dram_tensor("out", [L, 512], F32, kind="ExternalOutput")

    with tile.TileContext(nc) as tc:
        with tc.tile_pool(name="const", bufs=1) as constp, \
             tc.tile_pool(name="persist", bufs=1) as pers, \
             tc.tile_pool(name="dram", bufs=1, space="DRAM") as dramp, \
             tc.tile_pool(name="met", bufs=4) as metp, \
             tc.tile_pool(name="stage", bufs=1) as stagep, \
             tc.tile_pool(name="epool", bufs=3) as epool, \
             tc.tile_pool(name="oevac", bufs=2) as oevacp, \
             tc.tile_pool(name="rpool", bufs=4) as rpool, \
             tc.tile_pool(name="wpool", bufs=1) as wpool, \
             tc.tile_pool(name="xload", bufs=3) as xload, \
             tc.tile_pool(name="xtpool", bufs=1) as xtpool, \
             tc.tile_pool(name="mload", bufs=2) as mload, \
             tc.tile_pool(name="mtmp", bufs=2) as mtmp, \
             tc.tile_pool(name="spsum", bufs=2, space="PSUM") as spsum, \
             tc.tile_pool(name="opsum", bufs=2, space="PSUM") as opsum, \
             tc.tile_pool(name="scratch", bufs=2, space="PSUM") as scratch:

            # ---- constants
            idf32 = constp.tile([128, 128], F32, name="idf32", tag="idf32")
            make_identity(nc, idf32)
            idf32r = constp.tile([128, 128], F32R, name="idf32r", tag="idf32r")
            nc.vector.tensor_copy(idf32r, idf32)
            ones_f = constp.tile([1, 128], F32, name="ones_f", tag="ones_f")
            nc.vector.memset(ones_f, 1.0)
            ones_r = constp.tile([1, 128], F32R, name="ones_r", tag="ones_r")
            nc.vector.tensor_copy(ones_r, ones_f)

            # ---- persistent activations
            QT = pers.tile([128, NPAIR, L], F32R, name="QT", tag="QT")
            KT = pers.tile([128, NPAIR, L], F32R, name="KT", tag="KT")
            V = pers.tile([128, NKB, HPC, DH + 1], BF16, name="V", tag="V")
            nc.vector.memset(V[:, :, :, DH], 1.0)

            me_dram = dramp.tile([L, L], BF16, name="me_dram", tag="me_dram")

            # ---- weight / bias loads (DMA queue order matters: wk+wv feed
            # the pre-attention projections; masks for qb0 go next; wq is
            # only needed from qb1 on)

            xts = {}

            def xt_tile(lb):
                if lb not in xts:
                    xts[lb] = xtpool.tile([128, NDB, QBW], F32R,
                                          name="xt%d" % lb, tag="xt")
                return xts[lb]

            # ---------------- mask pipeline (half-width tiles) -----------
            def emit_mask_load(qb):
                mls = []
                for h in range(2):
                    ml = mload.tile([128, L // 2], F32,
                                    name=f"ml{qb}_{h}", tag="ml")
                    nc.sync.dma_start(
                        out=ml,
                        in_=mask_d[qb * 128:(qb + 1) * 128,
                                   h * (L // 2):(h + 1) * (L // 2)])
                    mls.append(ml)
                return mls

            def emit_mask_exp(qb, mls):
                mm = rpool.tile([128, 2], F32, name=f"mm{qb}", tag="rc")
                for h in range(2):
                    nc.vector.tensor_reduce(mm[:, h:h + 1], mls[h],
                                            axis=AX.X, op=ALU.min)
                mbias = rpool.tile([128, 1], F32, name=f"mb{qb}", tag="rc")
                nc.vector.tensor_reduce(mbias, mm, axis=AX.X, op=ALU.min)
                nc.vector.tensor_scalar_mul(mbias, mbias, -MASK_C)
                for h in range(2):
                    me = mtmp.tile([128, L // 2], BF16,
                                   name=f"me{qb}_{h}", tag="me")
                    nc.scalar.activation(me, mls[h], AF.Exp,
                                         bias=mbias, scale=MASK_C)
                    nc.sync.dma_start(
                        out=me_dram[qb * 128:(qb + 1) * 128,
                                    h * (L // 2):(h + 1) * (L // 2)],
                        in_=me)

            met_tiles = {}

            def emit_met(qb_, c):
                """Transposed M_e tiles for (q block, k-block chunk c)."""
                q0 = qb_ * QBW
                mh = metp.tile([128, 4, QBW], BF16,
                               name=f"met{qb_}_{c}", tag="met")
                met_tiles[(qb_, c)] = mh
                for i in range(4):
                    nc.sync.dma_start_transpose(
                        mh[:, i, :],
                        me_dram[q0:q0 + QBW,
                                (4 * c + i) * 128:(4 * c + i + 1) * 128])

            # ---------------- projection work items ---------------------
            xload_tiles = {}

            def emit_xt_load(lb, sh):
                for s in range(2):
                    xl = xload.tile([128, D], F32R,
                                    name=f"xl{lb}_{sh}_{s}", tag="xl")
                    nc.sync.dma_start(
                        out=xl,
                        in_=x_d[lb * 512 + (sh * 2 + s) * 128:
                                lb * 512 + (sh * 2 + s + 1) * 128, :])
                    xload_tiles[(lb, sh, s)] = xl

            def emit_xt(lb, sh):
                """Transpose X rows [lb*512+sh*256, +256) into xt."""
                xt = xt_tile(lb)
                if (lb, sh, 0) not in xload_tiles:
                    emit_xt_load(lb, sh)
                xls = [xload_tiles.pop((lb, sh, s)) for s in range(2)]
                for db in range(NDB):
                    tpt = scratch.tile([128, 256], F32R,
                                       name=f"tpd{lb}_{sh}_{db}", tag="sc")
                    for s in range(2):
                        nc.tensor.transpose(
                            tpt[:, s * 128:(s + 1) * 128],
                            xls[s][:, db * 128:(db + 1) * 128],
                            idf32r)
                    nc.vector.tensor_copy(
                        xt[:, db, sh * 256:(sh + 1) * 256], tpt)

            def emit_qk_chain(w_sb, bias_t, dst, np_, lb):
                """One projection chain into dst (KT slice or QT chunk)."""
                qp = scratch.tile([128, 512], F32,
                                  name=f"qp{np_}_{lb}_{id(w_sb) % 97}",
                                  tag="sc")
                for db in range(NDB):
                    nc.tensor.matmul(
                        qp,
                        w_sb[:, db, np_ * 128:(np_ + 1) * 128],
                        xt_tile(lb)[:, db, :],
                        start=(db == 0), stop=(db == NDB - 1))
                nc.vector.tensor_scalar_add(
                    dst[:, np_, lb * 512:(lb + 1) * 512], qp,
                    bias_t[:, np_:np_ + 1])

            def emit_v_chain(kb):
                vp = scratch.tile([128, 512], F32, name=f"vp{kb}", tag="sc")
                for db in range(NDB):
                    nc.tensor.matmul(
                        vp,
                        xt_tile(kb // 4)[:, db, (kb % 4) * 128:
                                         (kb % 4 + 1) * 128],
                        wv[:, db, :],
                        start=(db == 0), stop=False)
                nc.tensor.matmul(vp, ones_r[0:1, 0:128], bv,
                                 start=False, stop=True)
                nc.vector.tensor_copy(
                    V[:, kb, :, 0:DH],
                    vp.rearrange("p (h d) -> p h d", h=HPC))

            # ---------------- pump ---------------------------------------
            work = []

            def pump(n):
                for _ in range(n):
                    if work:
                        work.pop(0)()

            # ---------------- attention unit ------------------------------
            def emit_attn_pair(qb_, pr, rate=2):
                hA, hB = 2 * pr, 2 * pr + 1
                q0 = qb_ * QBW
                oa = opsum.tile([DH + 1, QBW], F32,
                                name=f"oa{qb_}_{pr}", tag="o")
                ob = opsum.tile([DH + 1, QBW], F32,
                                name=f"ob{qb_}_{pr}", tag="o")
                eps = {}

                def emit_ttpv(g):
                    """mask-multiply + PV for kb group (2g, 2g+1) — emitted
                    ~2 k-blocks late so the PE's in-order queue never parks
                    a PV (waiting on exp) in front of a ready QK."""
                    ppg = eps[g].rearrange("p a (b f) -> p a b f", b=2)
                    mh = met_tiles[(qb_, g // 2)]
                    base = mh[:, (2 * g) % 4, :]
                    mdup = bass.AP(
                        tensor=mh.tensor,
                        offset=base.offset,
                        ap=[mh.ap[0], [QBW, 2], [0, 2], [1, QBW]])
                    nc.vector.tensor_tensor(
                        out=ppg, in0=ppg, in1=mdup, op=ALU.mult)
                    for dkb in (2 * g, 2 * g + 1):
                        for o_ps, h, half in ((oa, hA, 0), (ob, hB, 1)):
                            nc.tensor.matmul(
                                o_ps,
                                V[:, dkb, h, :],
                                ppg[:, dkb % 2, half, :],
                                start=(dkb == 0),
                                stop=(dkb == NKB - 1))

                for kb in range(NKB):
                    if pr == 0 and kb == 0:
                        for c in range(4):
                            emit_met(qb_, c)
                    sp = spsum.tile([128, 1024], F32,
                                    name=f"sp{qb_}_{pr}_{kb}", tag="s")
                    nc.tensor.matmul(
                        sp[:, 0:512],
                        KT[0:64, pr, kb * 128:(kb + 1) * 128],
                        QT[0:64, pr, q0:q0 + QBW],
                        start=True, stop=True, tile_position=(0, 0))
                    nc.tensor.matmul(
                        sp[:, 512:1024],
                        KT[64:128, pr, kb * 128:(kb + 1) * 128],
                        QT[64:128, pr, q0:q0 + QBW],
                        start=True, stop=True, tile_position=(64, 0))
                    if kb % 2 == 0:
                        eps[kb // 2] = epool.tile(
                            [128, 2, 1024], BF16,
                            name=f"e{qb_}_{pr}_{kb}", tag="e")
                    nc.scalar.activation(eps[kb // 2][:, kb % 2, :],
                                         sp, AF.Exp)
                    if kb % 2 == 1:
                        pump(rate)
                        if kb >= 3:
                            emit_ttpv((kb - 3) // 2)
                def postproc():
                    stage = stagep.tile([128, 4, 128], F32,
                                        name=f"st{qb_}_{pr}", tag="st")
                    osbA = oevacp.tile([DH + 1, QBW], F32,
                                       name=f"oeA{qb_}_{pr}", tag="oe")
                    osbB = oevacp.tile([DH + 1, QBW], F32,
                                       name=f"oeB{qb_}_{pr}", tag="oe")
                    nc.vector.tensor_copy(osbA, oa)
                    nc.vector.tensor_copy(osbB, ob)
                    for osb, h in ((osbA, hA), (osbB, hB)):
                        hcol = (h % 2) * DH
                        tp = scratch.tile([128, 4 * 65], F32,
                                          name=f"tq{qb_}_{pr}_{h}", tag="sc")
                        for j in range(4):
                            nc.tensor.transpose(
                                tp[:, j * 65:(j + 1) * 65],
                                osb[:, j * 128:(j + 1) * 128],
                                idf32[0:65, 0:65])
                        tpv = tp.rearrange("p (j c) -> p j c", j=4)
                        rec = rpool.tile([128, 4], F32,
                                         name=f"rc{qb_}_{pr}_{h}", tag="rc")
                        nc.vector.reciprocal(rec, tpv[:, :, 64:65])
                        for j in range(4):
                            nc.vector.tensor_scalar_mul(
                                stage[:, j, hcol:hcol + DH],
                                tpv[:, j, 0:DH],
                                rec[:, j:j + 1])
                    nc.sync.dma_start(
                        out=out_d[q0:q0 + QBW, pr * 128:(pr + 1) * 128]
                        .rearrange("(j p) c -> p j c", p=128),
                        in_=stage)

                # defer the final PV group and the postproc to the next
                # unit's first pump call: its early QKs slot in ahead so
                # neither the PV tail nor the postproc transposes ever make
                # the ACT exp stream wait at a unit boundary
                work.insert(0, postproc)
                work.insert(0, lambda: emit_ttpv(NKB // 2 - 1))

            # ---------------- emission schedule ---------------------------
            # DMA queue order: X lb0 (gates everything), wk + bias tiles,
            # qb0's mask blocks (gate the met transposes), wq, wv.  The met
            # transposes for qb0 queue right behind the mask stores, ahead
            # of the pumped mask 4-15 loads.
            emit_xt_load(0, 0)
            emit_xt_load(0, 1)
            wk = wpool.tile([128, NDB, 512], F32R, name="wk", tag="wk")
            nc.sync.dma_start(out=wk,
                              in_=wk_d.rearrange("(c p) n -> p c n", p=128))
            bqt = wpool.tile([128, NPAIR], F32, name="bqt", tag="bqt")
            nc.sync.dma_start(out=bqt,
                              in_=bq_d.rearrange("o (c p) -> (o p) c", p=128))
            bkt = wpool.tile([128, NPAIR], F32, name="bkt", tag="bkt")
            nc.sync.dma_start(out=bkt,
                              in_=bk_d.rearrange("o (c p) -> (o p) c", p=128))
            emit_xt_load(1, 0)
            emit_xt_load(1, 1)
            wq = wpool.tile([128, NDB, 512], F32R, name="wq", tag="wq")
            nc.sync.dma_start(out=wq,
                              in_=wq_d.rearrange("(c p) n -> p c n", p=128))
            wv = wpool.tile([128, NDB, 512], F32R, name="wv", tag="wv")
            nc.sync.dma_start(out=wv,
                              in_=wv_d.rearrange("(c p) n -> p c n", p=128))
            bv = wpool.tile([1, 512], F32R, name="bv", tag="bv")
            nc.sync.dma_start(out=bv, in_=bv_d[:, :])
            mls0 = {qb: emit_mask_load(qb) for qb in range(4)}
            emit_xt(0, 0)
            emit_xt(0, 1)
            for qb in range(4):
                emit_mask_exp(qb, mls0.pop(qb))
            emit_qk_chain(wk, bkt, KT, 0, 0)
            emit_qk_chain(wq, bqt, QT, 0, 0)
            for kb in range(4):
                emit_v_chain(kb)

            def mask_item(qb):
                # exp scheduled a few pump slots after its load so the ACT
                # queue never head-blocks on an in-flight mask DMA
                mlq = emit_mask_load(qb)
                work.insert(min(8, len(work)),
                            lambda: emit_mask_exp(qb, mlq))

            def W(fn, *a):
                work.append(lambda: fn(*a))

            # Backlog drained in unit (0,0) (rate 9): the full projection in
            # l-chunk-major order (the rotating xt tile requires each chunk's
            # 12 consumers emitted before the chunk two slots later rebuilds)
            # plus the remaining mask pipelines, which feed the ACT engine
            # while the PE grinds through projection chains.
            proj = []
            proj += [(emit_qk_chain, wk, bkt, KT, pr, 0) for pr in (1, 2, 3)]
            proj += [(emit_qk_chain, wq, bqt, QT, pr, 0) for pr in (1, 2, 3)]
            late = []
            for lb in range(1, 4):
                if lb < 3:
                    proj.append((emit_xt_load, lb + 1, 0))
                    proj.append((emit_xt_load, lb + 1, 1))
                proj.append((emit_xt, lb, 0))
                proj.append((emit_xt, lb, 1))
                proj += [(emit_v_chain, kb)
                         for kb in range(4 * lb, 4 * lb + 4)]
                if lb < 3:
                    proj += [(emit_qk_chain, wk, bkt, KT, pr, lb)
                             for pr in range(4)]
                    proj += [(emit_qk_chain, wq, bqt, QT, pr, lb)
                             for pr in range(4)]
                else:
                    # the last l-chunk's xt is never overwritten, so its
                    # late-deadline chains can drain in later units' PE
                    # slack instead of stretching the PE-bound window:
                    # K p2/p3 l3 before units (0,2)/(0,3); Q l3 before qb3
                    proj += [(emit_qk_chain, wk, bkt, KT, pr, lb)
                             for pr in (0, 1)]
                    proj.append((emit_qk_chain, wq, bqt, QT, 0, lb))
                    late += [(emit_qk_chain, wk, bkt, KT, 2, lb),
                             (emit_qk_chain, wq, bqt, QT, 1, lb),
                             (emit_qk_chain, wk, bkt, KT, 3, lb),
                             (emit_qk_chain, wq, bqt, QT, 2, lb),
                             (emit_qk_chain, wq, bqt, QT, 3, lb)]
            mqueue = list(range(4, 16))
            for item in proj:
                W(*item)
            while mqueue:
                W(mask_item, mqueue.pop(0))
            for item in late:
                W(*item)

            emit_attn_pair(0, 0, rate=8)
            for pr in range(1, NPAIR):
                emit_attn_pair(0, pr, rate=2)
            for qb_ in range(1, NQB):
                for pr in range(NPAIR):
                    emit_attn_pair(qb_, pr, rate=2)
            pump(len(work))

    nc.finalize()
    return nc


def _get_nc():
    if "nc" not in _CACHE:
        _CACHE["nc"] = _build()
    return _CACHE["nc"]


def kernel(embedding, mask, Wq, bq, Wk, bk, Wv, bv):
    from concourse.bass_utils import run_bass_kernel_spmd

    nc = _get_nc()

    embedding = np.asarray(embedding, dtype=np.float32)
    mask = np.asarray(mask, dtype=np.float32)
    in_maps = []
    for c in range(NCORES):
        b = c // 2
        h0 = (c % 2) * HPC
        cs = slice(h0 * DH, (h0 + HPC) * DH)
        in_maps.append({
            "x": np.ascontiguousarray(embedding[b]),
            "mask": np.ascontiguousarray(mask[b, 0]),
            "wq": np.ascontiguousarray(np.asarray(Wq, np.float32)[:, cs]),
            "wk": np.ascontiguousarray(np.asarray(Wk, np.float32)[:, cs]),
            "wv": np.ascontiguousarray(np.asarray(Wv, np.float32)[:, cs]),
            "bq": np.ascontiguousarray(np.asarray(bq, np.float32)[cs]).reshape(1, 512),
            "bk": np.ascontiguousarray(np.asarray(bk, np.float32)[cs]).reshape(1, 512),
            "bv": np.ascontiguousarray(np.asarray(bv, np.float32)[cs]).reshape(1, 512),
        })

    res = run_bass_kernel_spmd(nc, in_maps, core_ids=list(range(NCORES)))

    out = np.empty((B, L, D), dtype=np.float32)
    for c in range(NCORES):
        b = c // 2
        h0 = (c % 2) * HPC
        out[b][:, h0 * DH:(h0 + HPC) * DH] = res.results[c]["out"]
    return out
